# revision 42
# baseline (speedup 1.0000x reference)
"""Trainium2 Bass kernel for nn_CombinedRepeatCausalLinear (two-phase bf16,
2-group pipelined scan).

Math: out[r, t] = sum_{s<=t} x[r, s] * (w0[s]*dv0^(t-s) + w1[t]*dv1^(t-s)) + bias[t]

Chunked linear attention with a matmul-computed hierarchical scan
(chunk L=128, 16 chunks, 2 groups of 8):

  Phase A -- per-chunk decayed sums U1_c / U0_c, accumulated into one
  psum bank per (group, half) by matmuls whose stationary places chunk
  c's sums at partitions 2*(c%8), 2*(c%8)+1.

  Scan -- one matmul per (group, half) against a triangular decay
  matrix turns the sums into per-chunk exclusive prefix states A1/A0
  (psum rows 0..15) plus, for group 0, the group-boundary state at
  psum rows 32..35 which is handed to group 1's sums tile (rows 32..35
  hi / 96..99 lo).  Group 1's phase B therefore only waits on its own
  sums, letting group 0's phase B and output DMA overlap group 1's
  input stream.

  Phase B -- per chunk: diagonal matmul + rank-2 cross matmul
  (contracting the group's state rows) accumulated in the same psum
  bank, then one psum->sbuf downcast copy and one output DMA.

Sums and states are kept as bf16 hi+lo pairs (lo = f32 - hi, computed
with quadrant-aligned DVE ops: hi rows base 0, lo rows base 64) so the
running state keeps ~fp32 precision.  A handful of dummy matmuls at
t=0 ramp the PE HAM clock-gate to full rate before the real work.

Everything on the device is bf16 (exact products, fp32 psum
accumulation).  The host ships x pre-transposed per shard in bf16,
upcasts the bf16 result to fp32, adds bias, and transposes back.
Data-parallel across 8 NeuronCores on the fused B*E axis.
"""

import sys

if "/opt/trn_rl_repo" not in sys.path:
    sys.path.insert(0, "/opt/trn_rl_repo")

import ml_dtypes
import numpy as np

import concourse.mybir as mybir
from concourse import bacc
from concourse.bass_utils import run_bass_kernel_spmd
from concourse.mybir import AluOpType
from concourse.tile import TileContext

_P = 128
_B, _E, _S = 4, 2048, 2048
_NCORES = 8
_R = (_B * _E) // _NCORES  # 1024 rows (r) per core
_L = 128  # chunk length along S
_NCH = _S // _L  # 16 chunks, exact
_G = 8  # chunks per group
_NG = _NCH // _G  # 2 groups
_HALF = 512  # r per matmul (one PSUM bank, fp32)
_NH = _R // _HALF  # 2 halves
_NWARM = 12  # PE clock-ramp dummy matmuls

_BF16 = mybir.dt.bfloat16
_F32 = mybir.dt.float32
_npbf16 = np.dtype(ml_dtypes.bfloat16)


def _build_host_mats(w0, w1, dv0, dv1):
    """Build Dall / DS / T (per group) / Md in float64, cast bf16."""
    w0 = w0.astype(np.float64)
    w1 = w1.astype(np.float64)
    sl = np.arange(_L)
    tl = np.arange(_L)
    diff = tl[None, :] - sl[:, None]
    mask = diff >= 0
    e = np.maximum(diff, 0)
    Dall = np.zeros((_P, _NCH * _P))
    DS = np.zeros((_P, 2 * _NCH))
    T = np.zeros((_P, _NG * _P))
    Md = np.zeros((_P, _NCH * _P))
    for c in range(_NCH):
        base = c * _L
        j = c % _G  # group-local chunk index
        Dall[:, c * _P : (c + 1) * _P] = np.where(
            mask,
            w0[base + sl][:, None] * (dv0**e) + w1[base + tl][None, :] * (dv1**e),
            0.0,
        )
        DS[:, 2 * c] = dv1 ** (_L - 1 - sl)
        DS[:, 2 * c + 1] = w0[base + sl] * dv0 ** (_L - 1 - sl)
        # cross stationary: hi state rows 2j, 2j+1; lo at 64+2j, 65+2j
        Md[2 * j, c * _P + tl] = Md[64 + 2 * j, c * _P + tl] = w1[base + tl] * dv1 ** (
            tl + 1
        )
        Md[2 * j + 1, c * _P + tl] = Md[65 + 2 * j, c * _P + tl] = dv0 ** (tl + 1)
    for g in range(_NG):
        Tg = T[:, g * _P : (g + 1) * _P]
        for j in range(_G):  # local state columns 2j, 2j+1
            for jp in range(j):
                d1 = dv1 ** ((j - 1 - jp) * _L)
                d0 = dv0 ** ((j - 1 - jp) * _L)
                Tg[2 * jp, 2 * j] = Tg[64 + 2 * jp, 2 * j] = d1
                Tg[2 * jp + 1, 2 * j + 1] = Tg[65 + 2 * jp, 2 * j + 1] = d0
            if g > 0:
                # boundary state enters via sums rows 32..35 (hi) 96..99 (lo)
                Tg[32, 2 * j] = Tg[96, 2 * j] = dv1 ** (j * _L)
                Tg[34, 2 * j + 1] = Tg[98, 2 * j + 1] = dv0 ** (j * _L)
        if g + 1 < _NG:
            # boundary-out columns 32..35 = [A1b, A1b, A0b, A0b]: full
            # prefix through this group's last chunk
            for jp in range(_G):
                d1 = dv1 ** ((_G - 1 - jp) * _L)
                d0 = dv0 ** ((_G - 1 - jp) * _L)
                Tg[2 * jp, 32] = Tg[2 * jp, 33] = d1
                Tg[64 + 2 * jp, 32] = Tg[64 + 2 * jp, 33] = d1
                Tg[2 * jp + 1, 34] = Tg[2 * jp + 1, 35] = d0
                Tg[65 + 2 * jp, 34] = Tg[65 + 2 * jp, 35] = d0
            if g > 0:
                Tg[32, 32] = Tg[32, 33] = Tg[96, 32] = Tg[96, 33] = dv1 ** (_G * _L)
                Tg[34, 34] = Tg[34, 35] = Tg[98, 34] = Tg[98, 35] = dv0 ** (_G * _L)
    cast = lambda a: a.astype(_npbf16)
    return cast(Dall), cast(DS), cast(T), cast(Md)


def _build():
    nc = bacc.Bacc(
        "TRN2",
        target_bir_lowering=False,
        debug=False,
        enable_asserts=False,
        num_devices=_NCORES,
    )
    xt = nc.dram_tensor("xt", [_S, _R], _BF16, kind="ExternalInput").ap()
    Dd = nc.dram_tensor("Dd", [_P, _NCH * _P], _BF16, kind="ExternalInput").ap()
    DSd = nc.dram_tensor("DSd", [_P, 2 * _NCH], _BF16, kind="ExternalInput").ap()
    Td = nc.dram_tensor("Td", [_P, _NG * _P], _BF16, kind="ExternalInput").ap()
    Md = nc.dram_tensor("Md", [_P, _NCH * _P], _BF16, kind="ExternalInput").ap()
    outT = nc.dram_tensor("outT", [_S, _R], _BF16, kind="ExternalOutput").ap()

    with TileContext(nc) as tc:
        with (
            tc.tile_pool(name="consts", bufs=1) as cpool,
            tc.tile_pool(name="xin", bufs=_NCH) as xpool,
            tc.tile_pool(name="ot", bufs=6) as otpool,
            tc.tile_pool(name="pacc", bufs=2, space="PSUM") as pspool,
            tc.tile_pool(name="pd", bufs=4, space="PSUM") as pdpool,
        ):
            # ---- PE clock-ramp warmup: result is never read ----
            warm = cpool.tile([_P, _HALF], _BF16)
            nc.gpsimd.memset(warm[:], 0.0)
            pwarm = pdpool.tile([_P, _HALF], _F32, tag="pd", name="pwarm")
            for _ in range(_NWARM):
                nc.tensor.matmul(
                    pwarm[:], warm[:, :_P], warm[:], start=True, stop=True
                )

            # ---- constants: scalar (ACT) hardware-DGE queue, before the
            # output transfers that share it ----
            DSt = cpool.tile([_P, 2 * _NCH], _BF16)
            nc.scalar.dma_start(DSt[:], DSd[:])
            Dall = cpool.tile([_P, _NCH * _P], _BF16)
            nc.scalar.dma_start(Dall[:], Dd[:])
            Tt = cpool.tile([_P, _NG * _P], _BF16)
            nc.scalar.dma_start(Tt[:], Td[:])
            Mall = cpool.tile([_P, _NCH * _P], _BF16)
            nc.scalar.dma_start(Mall[:], Md[:])

            # sum stationaries: zero tile, chunk c's two columns placed at
            # free offset c*128 + 2*(c%8) (-> psum partitions 2j, 2j+1)
            Dsum = cpool.tile([_P, _NCH * _P], _BF16)
            nc.gpsimd.memset(Dsum[:], 0.0)
            for c in range(_NCH - 1):  # last chunk's sum is never used
                o = c * _P + 2 * (c % _G)
                nc.vector.tensor_copy(Dsum[:, o : o + 2], DSt[:, 2 * c : 2 * c + 2])

            # ---- input: sync (SP) hardware-DGE queue, in chunk order ----
            xtiles = [
                xpool.tile([_P, _R], _BF16, tag="x", name=f"x{c}")
                for c in range(_NCH)
            ]
            for c in range(_NCH):
                nc.sync.dma_start(xtiles[c][:], xt[c * _L : (c + 1) * _L, :])

            # sums/states tiles are fully written by the widened hi/lo
            # copies below; only group 1's boundary-adjacent quadrants
            # need zeroing (the boundary rows are written mid-quadrant)
            sums = [None] * _NG
            states = [None] * _NG
            for g in range(_NG):
                sums[g] = cpool.tile([_P, _R], _BF16, name=f"sums{g}")
                states[g] = cpool.tile([_P, _R], _BF16, name=f"states{g}")
            for g in range(1, _NG):
                nc.gpsimd.memset(sums[g][32:64, :], 0.0)
                nc.gpsimd.memset(sums[g][96:128, :], 0.0)

            for g in range(_NG):
                c0, c1 = g * _G, (g + 1) * _G
                # ---- Phase A: chunk sums ----
                psg = [
                    pspool.tile([_P, _HALF], _F32, tag="acc", name=f"ps{g}{h}")
                    for h in range(_NH)
                ]
                lastc = min(c1 - 1, _NCH - 2)  # chunk 15's sum is never used
                order = list(range(c0, lastc + 1))
                for c in order:
                    for h in range(_NH):
                        nc.tensor.matmul(
                            psg[h][:],
                            Dsum[:, c * _P : (c + 1) * _P],
                            xtiles[c][:, h * _HALF : (h + 1) * _HALF],
                            start=(c == order[0]),
                            stop=(c == order[-1]),
                            skip_group_check=True,
                        )
                # widened hi/lo windows: psum rows beyond the real sums are
                # genuine zeros (stationary free size 128 writes all rows),
                # so copying them leaves the tile garbage-free without
                # memsets.  Group 1 keeps [32:64]/[96:128] for the boundary.
                nhi = 64 if g == 0 else 32
                for h in range(_NH):
                    cols = slice(h * _HALF, (h + 1) * _HALF)
                    nc.scalar.copy(sums[g][0:nhi, cols], psg[h][0:nhi, :])
                    nc.vector.tensor_tensor(
                        sums[g][64 : 64 + nhi, cols],
                        psg[h][0:nhi, :],
                        sums[g][0:nhi, cols],
                        AluOpType.subtract,
                    )

                # keep the PE busy (HAM clock-gate) while the scan's
                # psum->sbuf round trip runs on vector/scalar
                for _ in range(4):
                    nc.tensor.matmul(
                        pwarm[:], warm[:, :_P], warm[:], start=True, stop=True
                    )

                # ---- Scan ----
                for h in range(_NH):
                    cols = slice(h * _HALF, (h + 1) * _HALF)
                    pst = pspool.tile([_P, _HALF], _F32, tag="acc", name="pst")
                    nc.tensor.matmul(
                        pst[:],
                        Tt[:, g * _P : (g + 1) * _P],
                        sums[g][:, cols],
                        start=True,
                        stop=True,
                    )
                    nc.scalar.copy(states[g][0:64, cols], pst[0:64, :])
                    nc.vector.tensor_tensor(
                        states[g][64:128, cols],
                        pst[0:64, :],
                        states[g][0:64, cols],
                        AluOpType.subtract,
                    )
                    if g + 1 < _NG:
                        # boundary hand-off into next group's sums tile
                        nc.scalar.copy(sums[g + 1][32:36, cols], pst[32:36, :])
                        nc.vector.tensor_tensor(
                            sums[g + 1][96:100, cols],
                            pst[32:36, :],
                            sums[g + 1][32:36, cols],
                            AluOpType.subtract,
                        )

                # ---- Phase B ----
                for c in range(c0, c1):
                    ot = otpool.tile([_P, _R], _BF16, tag="ot", name="ot")
                    for h in range(_NH):
                        cols = slice(h * _HALF, (h + 1) * _HALF)
                        pd = pdpool.tile([_P, _HALF], _F32, tag="pd", name="pd")
                        nc.tensor.matmul(
                            pd[:],
                            Dall[:, c * _P : (c + 1) * _P],
                            xtiles[c][:, cols],
                            start=True,
                            stop=(c == 0),
                        )
                        if c > 0:
                            nc.tensor.matmul(
                                pd[:],
                                Mall[:, c * _P : (c + 1) * _P],
                                states[g][:, cols],
                                start=False,
                                stop=True,
                            )
                        if h == 0:
                            nc.vector.tensor_copy(ot[:, cols], pd[:])
                        else:
                            nc.scalar.copy(ot[:, cols], pd[:])
                    # last group's outputs ride the sync HWDGE queue (idle
                    # after the input stream) to cut the SWDGE drain tail
                    oeng = nc.gpsimd if g == 0 else nc.sync
                    oeng.dma_start(outT[c * _L : (c + 1) * _L, :], ot[:])
    nc.compile()
    return nc


def _run(x, weight, bias, decay_value, trace=False):
    x = np.asarray(x, dtype=np.float32)
    w = np.asarray(weight, dtype=np.float32)
    b = np.asarray(bias, dtype=np.float32)
    dv = np.asarray(decay_value, dtype=np.float32)
    dv0 = float(np.clip(dv[0, 0], 0.9, 1.0))
    dv1 = float(np.clip(dv[1, 0], 0.9, 1.0))

    Dall, DS, T, Md = _build_host_mats(w[0], w[1], dv0, dv1)
    nc = _build()

    xf = x.reshape(_B * _E, _S)
    xT = xf.T.astype(_npbf16)  # [S, B*E]
    in_maps = []
    for c in range(_NCORES):
        in_maps.append(
            {
                "xt": np.ascontiguousarray(xT[:, c * _R : (c + 1) * _R]),
                "Dd": Dall,
                "DSd": DS,
                "Td": T,
                "Md": Md,
            }
        )

    res = run_bass_kernel_spmd(nc, in_maps, core_ids=list(range(_NCORES)), trace=trace)
    outT = np.concatenate(
        [np.asarray(res.results[c]["outT"]) for c in range(_NCORES)], axis=1
    )  # [S, B*E] bf16
    full = np.ascontiguousarray(outT.T).astype(np.float32)
    if np.any(b):
        full += b[None, :]
    return full.reshape(_B, _E, _S), res


def kernel(x, weight, bias, decay_value):
    full, _ = _run(x, weight, bias, decay_value, trace=False)
    return full


# revision 44
# speedup vs baseline: 1.0393x; 1.0393x over previous
"""Trainium2 Bass kernel for nn_CombinedRepeatCausalLinear (two-phase bf16,
2-group pipelined scan).

Math: out[r, t] = sum_{s<=t} x[r, s] * (w0[s]*dv0^(t-s) + w1[t]*dv1^(t-s)) + bias[t]

Chunked linear attention with a matmul-computed hierarchical scan
(chunk L=128, 16 chunks, 2 groups of 8):

  Phase A -- per-chunk decayed sums U1_c / U0_c, accumulated into one
  psum bank per (group, half) by matmuls whose stationary places chunk
  c's sums at partitions 2*(c%8), 2*(c%8)+1.

  Scan -- one matmul per (group, half) against a triangular decay
  matrix turns the sums into per-chunk exclusive prefix states A1/A0
  (psum rows 0..15) plus, for group 0, the group-boundary state at
  psum rows 32..35 which is handed to group 1's sums tile (rows 32..35
  hi / 96..99 lo).  Group 1's phase B therefore only waits on its own
  sums, letting group 0's phase B and output DMA overlap group 1's
  input stream.

  Phase B -- per chunk: diagonal matmul + rank-2 cross matmul
  (contracting the group's state rows) accumulated in the same psum
  bank, then one psum->sbuf downcast copy and one output DMA.

Sums and states are kept as bf16 hi+lo pairs (lo = f32 - hi, computed
with quadrant-aligned DVE ops: hi rows base 0, lo rows base 64) so the
running state keeps ~fp32 precision.  A handful of dummy matmuls at
t=0 ramp the PE HAM clock-gate to full rate before the real work.

Everything on the device is bf16 (exact products, fp32 psum
accumulation).  The host ships x pre-transposed per shard in bf16,
upcasts the bf16 result to fp32, adds bias, and transposes back.
Data-parallel across 8 NeuronCores on the fused B*E axis.
"""

import sys

if "/opt/trn_rl_repo" not in sys.path:
    sys.path.insert(0, "/opt/trn_rl_repo")

import ml_dtypes
import numpy as np

import concourse.mybir as mybir
from concourse import bacc
from concourse.bass_utils import run_bass_kernel_spmd
from concourse.mybir import AluOpType
from concourse.tile import TileContext

_P = 128
_B, _E, _S = 4, 2048, 2048
_NCORES = 8
_R = (_B * _E) // _NCORES  # 1024 rows (r) per core
_L = 128  # chunk length along S
_NCH = _S // _L  # 16 chunks, exact
_G = 8  # chunks per group
_NG = _NCH // _G  # 2 groups
_HALF = 512  # r per matmul (one PSUM bank, fp32)
_NH = _R // _HALF  # 2 halves
_NWARM = 8  # PE clock-ramp dummy matmuls

_BF16 = mybir.dt.bfloat16
_F32 = mybir.dt.float32
_npbf16 = np.dtype(ml_dtypes.bfloat16)


def _build_host_mats(w0, w1, dv0, dv1):
    """Build Dall / DS / T (per group) / Md in float64, cast bf16."""
    w0 = w0.astype(np.float64)
    w1 = w1.astype(np.float64)
    sl = np.arange(_L)
    tl = np.arange(_L)
    diff = tl[None, :] - sl[:, None]
    mask = diff >= 0
    e = np.maximum(diff, 0)
    Dall = np.zeros((_P, _NCH * _P))
    DS = np.zeros((_P, 2 * _NCH))
    T = np.zeros((_P, _NG * _P))
    Md = np.zeros((_P, _NCH * _P))
    for c in range(_NCH):
        base = c * _L
        j = c % _G  # group-local chunk index
        Dall[:, c * _P : (c + 1) * _P] = np.where(
            mask,
            w0[base + sl][:, None] * (dv0**e) + w1[base + tl][None, :] * (dv1**e),
            0.0,
        )
        DS[:, 2 * c] = dv1 ** (_L - 1 - sl)
        DS[:, 2 * c + 1] = w0[base + sl] * dv0 ** (_L - 1 - sl)
        # cross stationary: hi state rows 2j, 2j+1; lo at 64+2j, 65+2j
        Md[2 * j, c * _P + tl] = Md[64 + 2 * j, c * _P + tl] = w1[base + tl] * dv1 ** (
            tl + 1
        )
        Md[2 * j + 1, c * _P + tl] = Md[65 + 2 * j, c * _P + tl] = dv0 ** (tl + 1)
    for g in range(_NG):
        Tg = T[:, g * _P : (g + 1) * _P]
        for j in range(_G):  # local state columns 2j, 2j+1
            for jp in range(j):
                d1 = dv1 ** ((j - 1 - jp) * _L)
                d0 = dv0 ** ((j - 1 - jp) * _L)
                Tg[2 * jp, 2 * j] = Tg[64 + 2 * jp, 2 * j] = d1
                Tg[2 * jp + 1, 2 * j + 1] = Tg[65 + 2 * jp, 2 * j + 1] = d0
            if g > 0:
                # boundary state enters via sums rows 32..35 (hi) 96..99 (lo)
                Tg[32, 2 * j] = Tg[96, 2 * j] = dv1 ** (j * _L)
                Tg[34, 2 * j + 1] = Tg[98, 2 * j + 1] = dv0 ** (j * _L)
        if g + 1 < _NG:
            # boundary-out columns 32..35 = [A1b, A1b, A0b, A0b]: full
            # prefix through this group's last chunk
            for jp in range(_G):
                d1 = dv1 ** ((_G - 1 - jp) * _L)
                d0 = dv0 ** ((_G - 1 - jp) * _L)
                Tg[2 * jp, 32] = Tg[2 * jp, 33] = d1
                Tg[64 + 2 * jp, 32] = Tg[64 + 2 * jp, 33] = d1
                Tg[2 * jp + 1, 34] = Tg[2 * jp + 1, 35] = d0
                Tg[65 + 2 * jp, 34] = Tg[65 + 2 * jp, 35] = d0
            if g > 0:
                Tg[32, 32] = Tg[32, 33] = Tg[96, 32] = Tg[96, 33] = dv1 ** (_G * _L)
                Tg[34, 34] = Tg[34, 35] = Tg[98, 34] = Tg[98, 35] = dv0 ** (_G * _L)
    cast = lambda a: a.astype(_npbf16)
    return cast(Dall), cast(DS), cast(T), cast(Md)


def _build():
    nc = bacc.Bacc(
        "TRN2",
        target_bir_lowering=False,
        debug=False,
        enable_asserts=False,
        num_devices=_NCORES,
    )
    xt = nc.dram_tensor("xt", [_S, _R], _BF16, kind="ExternalInput").ap()
    Dd = nc.dram_tensor("Dd", [_P, _NCH * _P], _BF16, kind="ExternalInput").ap()
    DSd = nc.dram_tensor("DSd", [_P, 2 * _NCH], _BF16, kind="ExternalInput").ap()
    Td = nc.dram_tensor("Td", [_P, _NG * _P], _BF16, kind="ExternalInput").ap()
    Md = nc.dram_tensor("Md", [_P, _NCH * _P], _BF16, kind="ExternalInput").ap()
    outT = nc.dram_tensor("outT", [_S, _R], _BF16, kind="ExternalOutput").ap()

    with TileContext(nc) as tc:
        with (
            tc.tile_pool(name="consts", bufs=1) as cpool,
            tc.tile_pool(name="xin", bufs=_NCH) as xpool,
            tc.tile_pool(name="ot", bufs=6) as otpool,
            tc.tile_pool(name="pacc", bufs=2, space="PSUM") as pspool,
            tc.tile_pool(name="pd", bufs=4, space="PSUM") as pdpool,
        ):
            # ---- PE clock-ramp warmup: result is never read ----
            warm = cpool.tile([_P, _HALF], _BF16)
            nc.gpsimd.memset(warm[:], 0.0)
            pwarm = pdpool.tile([_P, _HALF], _F32, tag="pd", name="pwarm")
            for _ in range(_NWARM):
                nc.tensor.matmul(
                    pwarm[:], warm[:, :_P], warm[:], start=True, stop=True
                )

            # ---- constants: scalar (ACT) hardware-DGE queue, before the
            # output transfers that share it ----
            DSt = cpool.tile([_P, 2 * _NCH], _BF16)
            nc.scalar.dma_start(DSt[:], DSd[:])
            Dall = cpool.tile([_P, _NCH * _P], _BF16)
            nc.scalar.dma_start(Dall[:], Dd[:])
            Tt = cpool.tile([_P, _NG * _P], _BF16)
            nc.scalar.dma_start(Tt[:], Td[:])
            Mall = cpool.tile([_P, _NCH * _P], _BF16)
            nc.scalar.dma_start(Mall[:], Md[:])

            # sum stationaries: zero tile, chunk c's two columns placed at
            # free offset c*128 + 2*(c%8) (-> psum partitions 2j, 2j+1)
            Dsum = cpool.tile([_P, _NCH * _P], _BF16)
            nc.gpsimd.memset(Dsum[:], 0.0)
            for c in range(_NCH - 1):  # last chunk's sum is never used
                o = c * _P + 2 * (c % _G)
                nc.vector.tensor_copy(Dsum[:, o : o + 2], DSt[:, 2 * c : 2 * c + 2])

            # ---- input: sync (SP) hardware-DGE queue, in chunk order ----
            xtiles = [
                xpool.tile([_P, _R], _BF16, tag="x", name=f"x{c}")
                for c in range(_NCH)
            ]
            for c in range(_NCH):
                nc.sync.dma_start(xtiles[c][:], xt[c * _L : (c + 1) * _L, :])

            # sums/states tiles are fully written by the widened hi/lo
            # copies below; only group 1's boundary-adjacent quadrants
            # need zeroing (the boundary rows are written mid-quadrant)
            sums = [None] * _NG
            states = [None] * _NG
            for g in range(_NG):
                sums[g] = cpool.tile([_P, _R], _BF16, name=f"sums{g}")
                states[g] = cpool.tile([_P, _R], _BF16, name=f"states{g}")
            for g in range(1, _NG):
                nc.gpsimd.memset(sums[g][32:64, :], 0.0)
                nc.gpsimd.memset(sums[g][96:128, :], 0.0)

            for g in range(_NG):
                c0, c1 = g * _G, (g + 1) * _G
                # ---- Phase A: chunk sums ----
                psg = [
                    pspool.tile([_P, _HALF], _F32, tag="acc", name=f"ps{g}{h}")
                    for h in range(_NH)
                ]
                lastc = min(c1 - 1, _NCH - 2)  # chunk 15's sum is never used
                order = list(range(c0, lastc + 1))
                for c in order:
                    for h in range(_NH):
                        nc.tensor.matmul(
                            psg[h][:],
                            Dsum[:, c * _P : (c + 1) * _P],
                            xtiles[c][:, h * _HALF : (h + 1) * _HALF],
                            start=(c == order[0]),
                            stop=(c == order[-1]),
                            skip_group_check=True,
                        )
                # widened hi/lo windows: psum rows beyond the real sums are
                # genuine zeros (stationary free size 128 writes all rows),
                # so copying them leaves the tile garbage-free without
                # memsets.  Group 1 keeps [32:64]/[96:128] for the boundary.
                nhi = 64 if g == 0 else 32
                for h in range(_NH):
                    cols = slice(h * _HALF, (h + 1) * _HALF)
                    nc.scalar.copy(sums[g][0:nhi, cols], psg[h][0:nhi, :])
                    nc.vector.tensor_tensor(
                        sums[g][64 : 64 + nhi, cols],
                        psg[h][0:nhi, :],
                        sums[g][0:nhi, cols],
                        AluOpType.subtract,
                    )

                # keep the PE busy (HAM clock-gate) while the scan's
                # psum->sbuf round trip runs on vector/scalar
                for _ in range(4):
                    nc.tensor.matmul(
                        pwarm[:], warm[:, :_P], warm[:], start=True, stop=True
                    )

                # ---- Scan ----
                for h in range(_NH):
                    cols = slice(h * _HALF, (h + 1) * _HALF)
                    pst = pspool.tile([_P, _HALF], _F32, tag="acc", name="pst")
                    nc.tensor.matmul(
                        pst[:],
                        Tt[:, g * _P : (g + 1) * _P],
                        sums[g][:, cols],
                        start=True,
                        stop=True,
                    )
                    nc.scalar.copy(states[g][0:64, cols], pst[0:64, :])
                    nc.vector.tensor_tensor(
                        states[g][64:128, cols],
                        pst[0:64, :],
                        states[g][0:64, cols],
                        AluOpType.subtract,
                    )
                    if g + 1 < _NG:
                        # boundary hand-off into next group's sums tile
                        nc.scalar.copy(sums[g + 1][32:36, cols], pst[32:36, :])
                        nc.vector.tensor_tensor(
                            sums[g + 1][96:100, cols],
                            pst[32:36, :],
                            sums[g + 1][32:36, cols],
                            AluOpType.subtract,
                        )

                # ---- Phase B ----
                for c in range(c0, c1):
                    ot = otpool.tile([_P, _R], _BF16, tag="ot", name="ot")
                    for h in range(_NH):
                        cols = slice(h * _HALF, (h + 1) * _HALF)
                        pd = pdpool.tile([_P, _HALF], _F32, tag="pd", name="pd")
                        nc.tensor.matmul(
                            pd[:],
                            Dall[:, c * _P : (c + 1) * _P],
                            xtiles[c][:, cols],
                            start=True,
                            stop=(c == 0),
                        )
                        if c > 0:
                            nc.tensor.matmul(
                                pd[:],
                                Mall[:, c * _P : (c + 1) * _P],
                                states[g][:, cols],
                                start=False,
                                stop=True,
                            )
                        if h == 0:
                            nc.vector.tensor_copy(ot[:, cols], pd[:])
                        else:
                            nc.scalar.copy(ot[:, cols], pd[:])
                    nc.gpsimd.dma_start(outT[c * _L : (c + 1) * _L, :], ot[:])
    nc.compile()
    return nc


def _run(x, weight, bias, decay_value, trace=False):
    x = np.asarray(x, dtype=np.float32)
    w = np.asarray(weight, dtype=np.float32)
    b = np.asarray(bias, dtype=np.float32)
    dv = np.asarray(decay_value, dtype=np.float32)
    dv0 = float(np.clip(dv[0, 0], 0.9, 1.0))
    dv1 = float(np.clip(dv[1, 0], 0.9, 1.0))

    Dall, DS, T, Md = _build_host_mats(w[0], w[1], dv0, dv1)
    nc = _build()

    xf = x.reshape(_B * _E, _S)
    xT = xf.T.astype(_npbf16)  # [S, B*E]
    in_maps = []
    for c in range(_NCORES):
        in_maps.append(
            {
                "xt": np.ascontiguousarray(xT[:, c * _R : (c + 1) * _R]),
                "Dd": Dall,
                "DSd": DS,
                "Td": T,
                "Md": Md,
            }
        )

    res = run_bass_kernel_spmd(nc, in_maps, core_ids=list(range(_NCORES)), trace=trace)
    outT = np.concatenate(
        [np.asarray(res.results[c]["outT"]) for c in range(_NCORES)], axis=1
    )  # [S, B*E] bf16
    full = np.ascontiguousarray(outT.T).astype(np.float32)
    if np.any(b):
        full += b[None, :]
    return full.reshape(_B, _E, _S), res


def kernel(x, weight, bias, decay_value):
    full, _ = _run(x, weight, bias, decay_value, trace=False)
    return full
